# revision 1
# baseline (speedup 1.0000x reference)
"""Trainium2 Bass kernel for nn_Head_72507637891886.

Computes r = exp(-(|k|_F^2+|q|_F^2)/2) * mean(cosh((k+q) @ w), -1) where
k = x@wk+bk, q = x@wq+bq, w = sqrt(32) * w_raw.T / |w_raw|_F.

Strategy: data-parallel over batch (2 batches = 8192 tokens per core, 8 cores).
Host pre-transposes each shard to [E=1024, 8192] so the contraction dim lands
on SBUF partitions; the fused [wk|wq] weight is the stationary operand.
Per 512-token block on device:
  - 8 accumulating matmuls (float32r, full rate) -> kq^T [64, 512] PSUM
  - ACT Identity+bias -> kqb (biased k,q, transposed)
  - DVE tensor_tensor_reduce -> per-feature sum-of-squares partial (|k|^2+|q|^2)
  - matmul with stacked [+wS | -wS] stationary -> [y^T; -y^T] [8, 512]
  - ACT Exp -> [e^y; e^-y], matmul with 0.125 const -> mean(cosh) [1, 512]
Host gathers, all-reduces the sum-of-squares scalar, applies the exp factor.
"""

import numpy as np

B, T, E, D = 16, 4096, 1024, 32
OMEGA = 4
NCORES = 8
TOK = B * T // NCORES  # 8192 tokens per core
BLK = 512              # tokens per block (matmul moving free dim)
NB = TOK // BLK        # 16 blocks
KC = E // 128          # 8 contraction chunks

_CACHE = {}
LAST_RESULTS = None  # BassKernelResults from the most recent run (for test.py)
LAST_PROFILE = None
LAST_OUTS = None
TRACE = False


def _build_bass():
    import concourse.bass as bass
    import concourse.mybir as mybir
    import concourse.tile as tile
    from concourse import bacc

    f32 = mybir.dt.float32
    f32r = mybir.dt.float32r
    AF = mybir.ActivationFunctionType

    nc = bacc.Bacc()
    xt = nc.declare_dram_parameter("xt", [E, TOK], f32r, isOutput=False)
    wkq = nc.declare_dram_parameter("wkq", [128, KC, 2 * D], f32r, isOutput=False)
    bkq = nc.declare_dram_parameter("bkq", [2 * D, 1], f32, isOutput=False)
    ws8 = nc.declare_dram_parameter("ws8", [2 * D, 2 * OMEGA], f32r, isOutput=False)
    c8 = nc.declare_dram_parameter("c8", [2 * OMEGA, 2], f32r, isOutput=False)
    rout = nc.declare_dram_parameter("rout", [1, TOK], f32, isOutput=True)
    ssout = nc.declare_dram_parameter("ssout", [2 * D, NB], f32, isOutput=True)

    with tile.TileContext(nc) as tc:
        with (
            tc.tile_pool(name="const", bufs=1) as const,
            tc.tile_pool(name="xp", bufs=3) as xp,
            tc.tile_pool(name="work", bufs=3) as work,
            tc.tile_pool(name="acc", bufs=1) as acc,
            tc.tile_pool(name="kqps", bufs=2, space="PSUM") as kqps,
            tc.tile_pool(name="yps", bufs=2, space="PSUM") as yps,
            tc.tile_pool(name="mps", bufs=2, space="PSUM") as mps,
        ):
            wkq_sb = const.tile([128, KC, 2 * D], f32r)
            nc.sync.dma_start(out=wkq_sb, in_=wkq[:])
            bkq_sb = const.tile([2 * D, 1], f32)
            nc.sync.dma_start(out=bkq_sb, in_=bkq[:])
            ws8_sb = const.tile([2 * D, 2 * OMEGA], f32r)
            nc.sync.dma_start(out=ws8_sb, in_=ws8[:])
            c8f = const.tile([2 * OMEGA, 2], f32r)
            nc.sync.dma_start(out=c8f, in_=c8[:])
            c8_sb = c8f[:, 0:1]     # 0.125 weights for the mean matmul
            zc8_sb = c8f[:, 1:2]    # 0.0 bias for the Exp activation

            ss_cols = acc.tile([2 * D, NB], f32)
            r_sb = acc.tile([1, TOK], f32)

            for ib in range(NB):
                tok = bass.ts(ib, BLK)
                x_tile = xp.tile([128, KC, BLK], f32r)
                nc.sync.dma_start(
                    out=x_tile,
                    in_=xt[:, tok].rearrange("(c p) t -> p c t", p=128),
                )

                kq_ps = kqps.tile([2 * D, BLK], f32)
                for c in range(KC):
                    nc.tensor.matmul(
                        kq_ps,
                        wkq_sb[:, c, :],
                        x_tile[:, c, :],
                        start=(c == 0),
                        stop=(c == KC - 1),
                    )

                # biased kq for the downstream matmul (sole consumer: PE)
                kqb = work.tile([2 * D, BLK], f32r)
                nc.scalar.activation(kqb, kq_ps, AF.Identity, bias=bkq_sb)
                # (k+bk)^2 and (q+bq)^2 summed along tokens via accum_out;
                # the squared tile itself is a write-only scratch.
                sq = work.tile([2 * D, BLK], f32, tag="sqdump")
                nc.scalar.activation(
                    sq, kq_ps, AF.Square, bias=bkq_sb,
                    accum_out=ss_cols[:, ib : ib + 1],
                )

                y8_ps = yps.tile([2 * OMEGA, BLK], f32)
                nc.tensor.matmul(y8_ps, ws8_sb, kqb, start=True, stop=True)

                e_sb = work.tile([2 * OMEGA, BLK], f32r)
                nc.scalar.activation(e_sb, y8_ps, AF.Exp, bias=zc8_sb)

                m_ps = mps.tile([1, BLK], f32)
                nc.tensor.matmul(m_ps, c8_sb, e_sb, start=True, stop=True)

                nc.scalar.activation(r_sb[:, tok], m_ps, AF.Copy)

            nc.sync.dma_start(out=rout[:], in_=r_sb)
            nc.sync.dma_start(out=ssout[:], in_=ss_cols)
    nc.compile()
    return nc


def _get_nc():
    if "nc" not in _CACHE:
        _CACHE["nc"] = _build_bass()
    return _CACHE["nc"]


def _run_profiled(nc, in_maps):
    """Run via PJRT with the NTFF profiler capturing; stash timing info in
    LAST_RESULTS-compatible globals."""
    global LAST_RESULTS, LAST_PROFILE
    import gauge.profiler
    from concourse import bass2jax

    prof = gauge.profiler.profile(
        kernel_dev_mode=True, profile_on_exit=False, bass_kernel=nc.m,
        fname="*",
    )
    with prof:
        results = bass2jax.run_bass_via_pjrt(nc, in_maps, n_cores=NCORES)
    LAST_PROFILE = prof
    LAST_RESULTS = None
    return results


def kernel(x, wq, bq, wk, bk, wv, bv, w_raw):
    global LAST_RESULTS
    from concourse.bass_utils import run_bass_kernel_spmd

    x = np.asarray(x, dtype=np.float32)
    wq = np.asarray(wq, dtype=np.float32)
    bq = np.asarray(bq, dtype=np.float32)
    wk = np.asarray(wk, dtype=np.float32)
    bk = np.asarray(bk, dtype=np.float32)
    w_raw = np.asarray(w_raw, dtype=np.float32)

    # replicated small operands
    wkq = np.concatenate([wk, wq], axis=1)  # [E, 64]
    wkq_sb = np.ascontiguousarray(
        wkq.reshape(KC, 128, 2 * D).transpose(1, 0, 2)
    )  # [128, KC, 64]
    bkq = np.ascontiguousarray(np.concatenate([bk, bq]).reshape(2 * D, 1))
    wt = w_raw.T.astype(np.float32)  # [D, OMEGA]
    norm = np.sqrt(np.sum(wt.astype(np.float32) ** 2, dtype=np.float32))
    w = (np.float32(np.sqrt(np.float32(D))) * (wt / norm)).astype(np.float32)
    wS = np.concatenate([w, w], axis=0)  # [64, OMEGA]
    ws8 = np.ascontiguousarray(np.concatenate([wS, -wS], axis=1))  # [64, 8]

    c8 = np.zeros((2 * OMEGA, 2), dtype=np.float32)
    c8[:, 0] = 0.125

    in_maps = []
    bpc = B // NCORES
    for c in range(NCORES):
        xt = np.ascontiguousarray(
            x[c * bpc : (c + 1) * bpc].reshape(TOK, E).T
        )  # [E, TOK]
        in_maps.append({"xt": xt, "wkq": wkq_sb, "bkq": bkq, "ws8": ws8, "c8": c8})

    global LAST_OUTS
    nc = _get_nc()
    res = run_bass_kernel_spmd(
        nc, in_maps, core_ids=list(range(NCORES)), trace=False
    )
    LAST_RESULTS = res
    results = res.results
    LAST_OUTS = results

    r_parts = []
    ss = 0.0
    for out in results:
        r_parts.append(out["rout"].reshape(TOK))
        ss += float(out["ssout"].sum(dtype=np.float64))

    with np.errstate(under="ignore"):
        a = np.float32(np.exp(np.float64(-ss / 2.0)))
    r = (a * np.concatenate(r_parts)).reshape(B, T).astype(np.float32)
    return r



# revision 11
# speedup vs baseline: 1.5442x; 1.5442x over previous
"""Trainium2 Bass kernel for nn_Head_72507637891886.

Computes r = exp(-(|k|_F^2+|q|_F^2)/2) * mean(cosh((k+q) @ w), -1) where
k = x@wk+bk, q = x@wq+bq, w = sqrt(32) * w_raw.T / |w_raw|_F.

Strategy: data-parallel over batch (2 batches = 8192 tokens per core, 8 cores).
The kernel is HBM-bound on streaming x (modeled 360 GB/s, fully serialized
across DMA queues), so x is shipped to the device as bf16 ([E, TOK] transposed
on host): halves the stream from 93us to 47us. The small matmul operands
(wkq, ws8, mean weights) are fused into one [128, 521] bf16 "wall" and the
f32 biases into a [64, 2] tensor, so two small DMAs precede the x stream.
Per block (15x512 + 2x256 tokens):
  - 8 accumulating bf16 matmuls -> kq^T [64, blk] PSUM f32
  - ACT Identity+bias -> kqb bf16 (k,q biased, transposed)
  - ACT Square+bias with accum_out -> per-feature sum-of-squares column
  - matmul with stacked [+wS | -wS] -> [y^T; -y^T] [8, blk] PSUM
  - ACT Exp -> [e^y; e^-y] bf16, matmul with 0.125 -> mean(cosh) [1, blk]
  - ACT Copy -> r row
Tail: rout [1, TOK] (SP queue) and ssout [64, NBLK] (ACT queue) ship in
parallel. Host all-reduces the sum-of-squares partials and applies the
exp(-z2/2) scale (underflows to 0 for this input scale).
"""

import numpy as np

B, T, E, D = 16, 4096, 1024, 32
OMEGA = 4
NCORES = 8
TOK = B * T // NCORES  # 8192 tokens per core
KC = E // 128          # 8 contraction chunks
BLOCKS = [512] * 15 + [256, 256]
NBLK = len(BLOCKS)
WALL_F = 521           # 512 wkq | 8 ws8 | 1 mean-w

_CACHE = {}
LAST_RESULTS = None  # BassKernelResults from the most recent run (for test.py)
LAST_PROFILE = None
LAST_OUTS = None
TRACE = False


def _build_bass():
    import concourse.bass as bass
    import concourse.mybir as mybir
    import concourse.tile as tile
    from concourse import bacc

    f32 = mybir.dt.float32
    f32r = mybir.dt.float32r
    bf16 = mybir.dt.bfloat16
    AF = mybir.ActivationFunctionType
    ALU = mybir.AluOpType

    nc = bacc.Bacc()
    xt = nc.declare_dram_parameter("xt", [E, TOK], bf16, isOutput=False)
    wall = nc.declare_dram_parameter("wall", [128, WALL_F], bf16, isOutput=False)
    bias3 = nc.declare_dram_parameter("bias3", [2 * D, 2], f32, isOutput=False)
    rout = nc.declare_dram_parameter("rout", [1, TOK], f32, isOutput=True)
    ssout = nc.declare_dram_parameter("ssout", [2 * D, NBLK], f32, isOutput=True)

    with tile.TileContext(nc) as tc:
        with (
            tc.tile_pool(name="const", bufs=1) as const,
            tc.tile_pool(name="xp", bufs=3) as xp,
            tc.tile_pool(name="work", bufs=3) as work,
            tc.tile_pool(name="acc", bufs=1) as acc,
            tc.tile_pool(name="kqps", bufs=2, space="PSUM") as kqps,
            tc.tile_pool(name="yps", bufs=2, space="PSUM") as yps,
            tc.tile_pool(name="mps", bufs=2, space="PSUM") as mps,
        ):
            wall_sb = const.tile([128, WALL_F], bf16)
            nc.sync.dma_start(out=wall_sb, in_=wall[:])
            bias3_sb = const.tile([2 * D, 2], f32)
            nc.sync.dma_start(out=bias3_sb, in_=bias3[:])
            wkq_sb = wall_sb[:, 0:512]
            ws8_sb = wall_sb[0:64, 512:520]
            c8w_sb = wall_sb[0:8, 520:521]
            bkq_sb = bias3_sb[0:64, 0:1]
            zero8_sb = bias3_sb[0:8, 1:2]

            ss_cols = acc.tile([2 * D, NBLK], f32)
            r_sb = acc.tile([1, TOK], f32)

            t0 = 0
            for ib, blk in enumerate(BLOCKS):
                x_tile = xp.tile([128, KC, blk], bf16, tag="x")
                nc.sync.dma_start(
                    out=x_tile,
                    in_=xt[:, t0 : t0 + blk].rearrange("(c p) t -> p c t", p=128),
                )

                kq_ps = kqps.tile([2 * D, blk], f32, tag="kq")
                for c in range(KC):
                    nc.tensor.matmul(
                        kq_ps,
                        wkq_sb[:, c * 64 : (c + 1) * 64],
                        x_tile[:, c, :],
                        start=(c == 0),
                        stop=(c == KC - 1),
                    )

                # biased kq for the two downstream matmuls
                kqb = work.tile([2 * D, blk], bf16, tag="kqb")
                nc.scalar.activation(kqb, kq_ps, AF.Identity, bias=bkq_sb)

                # (k+bk)^2 and (q+bq)^2 summed along tokens via accum_out;
                # the squared tile itself is a write-only scratch.
                sq = work.tile([2 * D, blk], f32, tag="sq")
                nc.scalar.activation(
                    sq, kq_ps, AF.Square, bias=bkq_sb,
                    accum_out=ss_cols[:, ib : ib + 1],
                )

                y8_ps = yps.tile([2 * OMEGA, blk], f32, tag="y8")
                nc.tensor.matmul(y8_ps, ws8_sb, kqb, start=True, stop=True)

                e_sb = work.tile([2 * OMEGA, blk], bf16, tag="e")
                nc.scalar.activation(e_sb, y8_ps, AF.Exp, bias=zero8_sb)

                m_ps = mps.tile([1, blk], f32, tag="m")
                nc.tensor.matmul(m_ps, c8w_sb, e_sb, start=True, stop=True)

                nc.scalar.activation(r_sb[:, t0 : t0 + blk], m_ps, AF.Copy)
                t0 += blk

            # tail outputs on separate HWDGE queues so their DGE stages overlap
            nc.sync.dma_start(out=rout[:], in_=r_sb)
            nc.scalar.dma_start(out=ssout[:], in_=ss_cols)
    nc.compile()
    return nc


def _get_nc():
    if "nc" not in _CACHE:
        _CACHE["nc"] = _build_bass()
    return _CACHE["nc"]


def _make_inputs(x, wq, bq, wk, bk, w_raw):
    import ml_dtypes

    bf16 = ml_dtypes.bfloat16
    # replicated small operands, fused into one [128, WALL_F] bf16 wall
    wkq = np.concatenate([wk, wq], axis=1)  # [E, 64]
    wkq_p = wkq.reshape(KC, 128, 2 * D).transpose(1, 0, 2).reshape(128, 512)
    wt = w_raw.T.astype(np.float32)  # [D, OMEGA]
    norm = np.sqrt(np.sum(wt ** 2, dtype=np.float32))
    w = (np.float32(np.sqrt(np.float32(D))) * (wt / norm)).astype(np.float32)
    wS = np.concatenate([w, w], axis=0)  # [64, OMEGA]
    ws8 = np.concatenate([wS, -wS], axis=1)  # [64, 8]

    wall = np.zeros((128, WALL_F), dtype=np.float32)
    wall[:, 0:512] = wkq_p
    wall[0:64, 512:520] = ws8
    wall[0:8, 520] = 0.125
    wall_b = wall.astype(bf16)

    bias3 = np.zeros((2 * D, 2), dtype=np.float32)
    bias3[:, 0] = np.concatenate([bk, bq])

    in_maps = []
    bpc = B // NCORES
    for c in range(NCORES):
        xt = np.ascontiguousarray(
            x[c * bpc : (c + 1) * bpc].reshape(TOK, E).astype(bf16).T
        )  # [E, TOK] bf16
        in_maps.append({"xt": xt, "wall": wall_b, "bias3": bias3})
    return in_maps


def kernel(x, wq, bq, wk, bk, wv, bv, w_raw):
    global LAST_RESULTS, LAST_OUTS
    from concourse.bass_utils import run_bass_kernel_spmd

    x = np.asarray(x, dtype=np.float32)
    wq = np.asarray(wq, dtype=np.float32)
    bq = np.asarray(bq, dtype=np.float32)
    wk = np.asarray(wk, dtype=np.float32)
    bk = np.asarray(bk, dtype=np.float32)
    w_raw = np.asarray(w_raw, dtype=np.float32)

    in_maps = _make_inputs(x, wq, bq, wk, bk, w_raw)

    nc = _get_nc()
    res = run_bass_kernel_spmd(
        nc, in_maps, core_ids=list(range(NCORES)), trace=False
    )
    LAST_RESULTS = res
    results = res.results
    LAST_OUTS = results

    r_parts = []
    ss = 0.0
    for out in results:
        r_parts.append(out["rout"].reshape(TOK))
        ss += float(out["ssout"].sum(dtype=np.float64))

    with np.errstate(under="ignore"):
        a = np.float32(np.exp(np.float64(-ss / 2.0)))
    r = (a * np.concatenate(r_parts)).reshape(B, T).astype(np.float32)
    return r


# revision 15
# speedup vs baseline: 1.6172x; 1.0472x over previous
"""Trainium2 Bass kernel for nn_Head_72507637891886.

Computes r = exp(-(|k|_F^2+|q|_F^2)/2) * mean(cosh((k+q) @ w), -1) where
k = x@wk+bk, q = x@wq+bq, w = sqrt(32) * w_raw.T / |w_raw|_F.

Strategy: data-parallel over batch (2 batches = 8192 tokens per core, 8 cores).
The kernel is HBM-bound on streaming x (modeled 360 GB/s, fully serialized
across DMA queues), so x is shipped to the device as bf16 ([E, TOK] transposed
on host): halves the stream from 93us to 47us. The small matmul operands
(wkq, ws8, mean weights) are fused into one [128, 521] bf16 "wall" and the
f32 biases into a [64, 2] tensor, so two small DMAs precede the x stream.
Per block (15x512 + 2x256 tokens), work split so no engine exceeds the
2.9us DMA period (ACT ~1.4us, DVE ~1.4us, PE ~2.4us):
  - 8 accumulating bf16 matmuls -> kq^T [64, blk] PSUM f32
  - DVE tensor_scalar_add(+bias) -> kqb bf16 (k,q biased, transposed)
  - ACT Square+bias with accum_out -> per-feature sum-of-squares column
  - matmul with stacked [+wS | -wS] -> [y^T; -y^T] [8, blk] PSUM
  - ACT Exp -> [e^y; e^-y] bf16, matmul with 0.125 -> mean(cosh) [1, blk]
  - DVE tensor_scalar_add(+0) -> r row
Tail: rout [1, TOK] (SP queue) and ssout [64, NBLK] (ACT queue) ship in
parallel. Host all-reduces the sum-of-squares partials and applies the
exp(-z2/2) scale (underflows to 0 for this input scale).
"""

import numpy as np

B, T, E, D = 16, 4096, 1024, 32
OMEGA = 4
NCORES = 8
TOK = B * T // NCORES  # 8192 tokens per core
KC = E // 128          # 8 contraction chunks
BLOCKS = [512] * 15 + [256, 256]
NBLK = len(BLOCKS)
WALL_F = 521           # 512 wkq | 8 ws8 | 1 mean-w

_CACHE = {}
LAST_RESULTS = None  # BassKernelResults from the most recent run (for test.py)
LAST_PROFILE = None
LAST_OUTS = None
TRACE = False


def _build_bass():
    import concourse.bass as bass
    import concourse.mybir as mybir
    import concourse.tile as tile
    from concourse import bacc

    f32 = mybir.dt.float32
    f32r = mybir.dt.float32r
    bf16 = mybir.dt.bfloat16
    AF = mybir.ActivationFunctionType
    ALU = mybir.AluOpType

    nc = bacc.Bacc()
    xt = nc.declare_dram_parameter("xt", [E, TOK], bf16, isOutput=False)
    wall = nc.declare_dram_parameter("wall", [128, WALL_F], bf16, isOutput=False)
    bias3 = nc.declare_dram_parameter("bias3", [2 * D, 2], f32, isOutput=False)
    rout = nc.declare_dram_parameter("rout", [1, TOK], f32, isOutput=True)
    ssout = nc.declare_dram_parameter("ssout", [2 * D, NBLK], f32, isOutput=True)

    with tile.TileContext(nc) as tc:
        with (
            tc.tile_pool(name="const", bufs=1) as const,
            tc.tile_pool(name="xp", bufs=4) as xp,
            tc.tile_pool(name="work", bufs=3) as work,
            tc.tile_pool(name="acc", bufs=1) as acc,
            tc.tile_pool(name="kqps", bufs=3, space="PSUM") as kqps,
            tc.tile_pool(name="yps", bufs=2, space="PSUM") as yps,
            tc.tile_pool(name="mps", bufs=2, space="PSUM") as mps,
        ):
            wall_sb = const.tile([128, WALL_F], bf16)
            nc.sync.dma_start(out=wall_sb, in_=wall[:])
            bias3_sb = const.tile([2 * D, 2], f32)
            nc.sync.dma_start(out=bias3_sb, in_=bias3[:])
            wkq_sb = wall_sb[:, 0:512]
            ws8_sb = wall_sb[0:64, 512:520]
            c8w_sb = wall_sb[0:8, 520:521]
            bkq_sb = bias3_sb[0:64, 0:1]
            zero8_sb = bias3_sb[0:8, 1:2]

            ss_cols = acc.tile([2 * D, NBLK], f32)
            r_sb = acc.tile([1, TOK], f32)

            t0 = 0
            for ib, blk in enumerate(BLOCKS):
                x_tile = xp.tile([128, KC, blk], bf16, tag="x")
                nc.sync.dma_start(
                    out=x_tile,
                    in_=xt[:, t0 : t0 + blk].rearrange("(c p) t -> p c t", p=128),
                )

                kq_ps = kqps.tile([2 * D, blk], f32, tag="kq")
                for c in range(KC):
                    nc.tensor.matmul(
                        kq_ps,
                        wkq_sb[:, c * 64 : (c + 1) * 64],
                        x_tile[:, c, :],
                        start=(c == 0),
                        stop=(c == KC - 1),
                    )

                # biased kq for the two downstream matmuls (DVE copy+bias)
                kqb = work.tile([2 * D, blk], bf16, tag="kqb")
                nc.vector.tensor_scalar_add(kqb, kq_ps, bkq_sb)

                # (k+bk)^2 and (q+bq)^2 summed along tokens via accum_out;
                # the squared tile itself is a write-only scratch.
                sq = work.tile([2 * D, blk], f32, tag="sq")
                nc.scalar.activation(
                    sq, kq_ps, AF.Square, bias=bkq_sb,
                    accum_out=ss_cols[:, ib : ib + 1],
                )

                y8_ps = yps.tile([2 * OMEGA, blk], f32, tag="y8")
                nc.tensor.matmul(y8_ps, ws8_sb, kqb, start=True, stop=True)

                e_sb = work.tile([2 * OMEGA, blk], bf16, tag="e")
                nc.scalar.activation(e_sb, y8_ps, AF.Exp, bias=zero8_sb)

                m_ps = mps.tile([1, blk], f32, tag="m")
                nc.tensor.matmul(m_ps, c8w_sb, e_sb, start=True, stop=True)

                nc.vector.tensor_scalar_add(r_sb[:, t0 : t0 + blk], m_ps, 0.0)
                t0 += blk

            # tail outputs on separate HWDGE queues so their DGE stages overlap
            nc.sync.dma_start(out=rout[:], in_=r_sb)
            nc.scalar.dma_start(out=ssout[:], in_=ss_cols)
    nc.compile()
    return nc


def _get_nc():
    if "nc" not in _CACHE:
        _CACHE["nc"] = _build_bass()
    return _CACHE["nc"]


def _make_inputs(x, wq, bq, wk, bk, w_raw):
    import ml_dtypes

    bf16 = ml_dtypes.bfloat16
    # replicated small operands, fused into one [128, WALL_F] bf16 wall
    wkq = np.concatenate([wk, wq], axis=1)  # [E, 64]
    wkq_p = wkq.reshape(KC, 128, 2 * D).transpose(1, 0, 2).reshape(128, 512)
    wt = w_raw.T.astype(np.float32)  # [D, OMEGA]
    norm = np.sqrt(np.sum(wt ** 2, dtype=np.float32))
    w = (np.float32(np.sqrt(np.float32(D))) * (wt / norm)).astype(np.float32)
    wS = np.concatenate([w, w], axis=0)  # [64, OMEGA]
    ws8 = np.concatenate([wS, -wS], axis=1)  # [64, 8]

    wall = np.zeros((128, WALL_F), dtype=np.float32)
    wall[:, 0:512] = wkq_p
    wall[0:64, 512:520] = ws8
    wall[0:8, 520] = 0.125
    wall_b = wall.astype(bf16)

    bias3 = np.zeros((2 * D, 2), dtype=np.float32)
    bias3[:, 0] = np.concatenate([bk, bq])

    in_maps = []
    bpc = B // NCORES
    for c in range(NCORES):
        xt = np.ascontiguousarray(
            x[c * bpc : (c + 1) * bpc].reshape(TOK, E).astype(bf16).T
        )  # [E, TOK] bf16
        in_maps.append({"xt": xt, "wall": wall_b, "bias3": bias3})
    return in_maps


def kernel(x, wq, bq, wk, bk, wv, bv, w_raw):
    global LAST_RESULTS, LAST_OUTS
    from concourse.bass_utils import run_bass_kernel_spmd

    x = np.asarray(x, dtype=np.float32)
    wq = np.asarray(wq, dtype=np.float32)
    bq = np.asarray(bq, dtype=np.float32)
    wk = np.asarray(wk, dtype=np.float32)
    bk = np.asarray(bk, dtype=np.float32)
    w_raw = np.asarray(w_raw, dtype=np.float32)

    in_maps = _make_inputs(x, wq, bq, wk, bk, w_raw)

    nc = _get_nc()
    res = run_bass_kernel_spmd(
        nc, in_maps, core_ids=list(range(NCORES)), trace=False
    )
    LAST_RESULTS = res
    results = res.results
    LAST_OUTS = results

    r_parts = []
    ss = 0.0
    for out in results:
        r_parts.append(out["rout"].reshape(TOK))
        ss += float(out["ssout"].sum(dtype=np.float64))

    with np.errstate(under="ignore"):
        a = np.float32(np.exp(np.float64(-ss / 2.0)))
    r = (a * np.concatenate(r_parts)).reshape(B, T).astype(np.float32)
    return r
